# revision 72
# baseline (speedup 1.0000x reference)
"""Dense-CRF relaxed Potts loss on 8 TRN2 NeuronCores — symmetric-p version.

Math: every off-diagonal slab-pair block (a,b) contributes
0.5*sum(W) - 2*p^T W p with p = s - 1/2 (identity: s_i(1-s_j)+(1-s_i)s_j =
1/2 - 2 p_i p_j), where W = exp(-0.5*d2) is the raw Gaussian affinity.
The N x N triangle is processed as 324 blocks of 128x128 per core (9 own
slabs x cyclic offsets d=1..36); d=36 pairs are computed by both owners and
the host subtracts one exact copy; d=0 self blocks are exact on the host.

Engine split (all ~75% busy, exp-limited):
  - PE: z = f_i.f_j - 0.5sq_i - 0.5sq_j via a K=36 3-limb bf16 matmul (the
    row term rides the matmul so activations need no per-slab bias and can
    span slab boundaries), plus the p^T(WP) / sum(W~) contractions into
    column-folded PSUM accumulators.  A zero dummy matmul at t~0 pins
    pe_busy_start so everything runs at the fully-ramped PE rate.
  - ACT: exp for blocks with m = 8t+d < M0 (bias 0), with accum_out giving
    the sum(W) row sums for free; 12-block instructions amortize the
    185ns access bubble + 187ns accumulator read.
  - DVE: the W*p_j multiply for every block (bf16 2x mode), and for the
    m >= M0 blocks the exp itself via an int16 Schraudolph bitcast:
    u = rne_i16(z*128*log2e + (127+71)*128 - 7.335) bitcast to bf16 gives
    W*2^71 (the +71 exponent shift keeps all z in [-137,0] positive-
    exponent; scaled P columns and a 2^-71 ones-lhsT descale the sums).
    Those blocks run in a dedicated 1-bank PSUM pool (4-block groups
    interleaved between ACT groups) so the slow 1x psum-read cvt never
    blocks the ACT pipeline's 3+3-bank double buffer.
Contractions lag their group by LAG so a late P broadcast never head-of-
line-blocks PE; the final two groups (one per path) skip reduction and are
DMA'd raw for host-side reduction, shortening the device tail.
"""

import numpy as np
import ml_dtypes

import concourse.bacc as bacc
import concourse.tile as tile
from concourse import mybir
import concourse.bass_utils as bass_utils

BF16 = ml_dtypes.bfloat16

SIGMA_XY = 15.0
SIGMA_RGB = 0.125
H = W = 96
N = H * W                   # 9216
N_CORES = 8
NSLAB = N // 128            # 72 slabs of 128 rows
T_SLABS = NSLAB // N_CORES  # 9 own slabs per core
D_MAX = 36
BEXT = (8 * (T_SLABS - 1) + D_MAX + 1) * 128   # 12928 extended b columns
GROUP_CAPS = (12, 12)       # act groups double-buffer in 3+3 psum banks
DVE_CAP = 4                 # dve groups use a dedicated 1-bank psum pool
M0 = 77                     # blocks with m = 8t+d >= M0 take the DVE exp path
SC = 71.0                   # DVE-path scale: W~ = W * 2^SC (bf16 bitcast exp)
LOG2E = 1.4426950408889634
CVT_C1 = float(np.float32(128.0 * LOG2E))
CVT_C2 = float(np.float32((127.0 + SC) * 128.0 - 7.335))

_cached = {}


def _plan():
    """Typed group schedule: list of (gi, nb, parity, segments, typ) where
    segments are (t, d0, nblk, tile_off) runs and typ is 'act' or 'dve'.
    Blocks with m = 8t+d >= M0 use the DVE i16-exp path; runs are ordered so
    same-type runs merge, and the trailing DVE groups end small (the final
    4-block group is the host dump)."""
    # act stream: blocks in (t, d) order, packed [4, 12, 12, ..., 12, 4]
    act_blocks = []
    dve_groups = []         # 4-block groups
    for t in range(T_SLABS):
        cut = max(1, M0 - 8 * t)        # d >= cut -> dve
        act_blocks += [(t, d) for d in range(1, min(D_MAX + 1, cut))]
        for d0 in range(cut, D_MAX + 1, DVE_CAP):
            dve_groups.append([(t, d0, DVE_CAP, 0)])
    assert len(act_blocks) % 4 == 0

    def segs_of(blks):
        segs = []
        off = 0
        for (t, d) in blks:
            if segs and segs[-1][0] == t and segs[-1][1] + segs[-1][2] == d:
                segs[-1][2] += 1
            else:
                segs.append([t, d, 1, off])
            off += 128
        return [tuple(x) for x in segs]

    n_ab = len(act_blocks)
    act_groups = [segs_of(act_blocks[0:4])]
    pos = 4
    while n_ab - pos > 16:
        act_groups.append(segs_of(act_blocks[pos:pos + 12]))
        pos += 12
    if n_ab - pos > 4:
        act_groups.append(segs_of(act_blocks[pos:n_ab - 4]))
        pos = n_ab - 4
    act_groups.append(segs_of(act_blocks[pos:]))

    # interleave: dve groups after act group IL0, one per act group; the last
    # dve group (host dump) goes second-to-last, the final 4-block act group
    # (also host-dumped) last
    IL0 = 10
    seq = []                # (segs, typ)
    di = 0
    for ai, segs in enumerate(act_groups[:-1]):
        seq.append((segs, "act"))
        if ai >= IL0 and di < len(dve_groups) - 1:
            seq.append(([tuple(x) for x in dve_groups[di]], "dve"))
            di += 1
    while di < len(dve_groups):
        seq.append(([tuple(x) for x in dve_groups[di]], "dve"))
        di += 1
    seq.append((act_groups[-1], "act"))

    plan = []
    n_act = 0
    for gi, (segs, typ) in enumerate(seq):
        nb = sum(s[2] for s in segs)
        if typ == "act":
            parity = n_act % 2
            n_act += 1
            assert nb <= GROUP_CAPS[parity]
        else:
            parity = 2
            assert nb <= DVE_CAP
        plan.append((gi, nb, parity, segs, typ))
    assert sum(nb for (_, nb, _, _, _) in plan) == 324
    assert plan[-1][1] == 4 and plan[-1][4] == "act"
    assert plan[-2][1] == 4 and plan[-2][4] == "dve"
    return plan


def _chunks(segs, cell=512):
    """Cell-aligned psum chunks: (t, bcol, tile_off, w) split at segment and
    cell boundaries (matmul output must stay within one psum bank / fold)."""
    out = []
    for (t, d0, nblk, off) in segs:
        w = nblk * 128
        bcol = (8 * t + d0) * 128
        rel = 0
        while rel < w:
            cell_end = ((off + rel) // cell + 1) * cell
            cw = min(w - rel, cell_end - (off + rel))
            out.append((t, bcol + rel, off + rel, cw))
            rel += cw
    return out


def _pslices(segs):
    """P-tile slices for the DVE multiply: (tile_off, pcol, w).  P is the
    extended [128, BEXT] broadcast (no wrap: phys col m <= 100 < BEXT/128)."""
    return [(off, (8 * t + d0) * 128, nblk * 128) for (t, d0, nblk, off) in segs]


def _build_module():
    plan = _plan()
    ng = len(plan)
    n_dump = 2                            # last two groups are host-reduced
    n_accum = sum(1 for g in plan[:-1] if g[4] == "act")
    f32 = mybir.dt.float32
    bf = mybir.dt.bfloat16
    i16 = mybir.dt.int16

    # start/stop counts for the two M2 accumulation chains
    n_conA = sum(len(_chunks(segs)) for (gi, nb, par, segs, typ) in plan
                 if gi < ng - n_dump)
    n_swA = sum(len(_chunks(segs, 256)) for (gi, nb, par, segs, typ) in plan
                if typ == "dve" and gi < ng - n_dump)

    nc = bacc.Bacc(
        "TRN2",
        target_bir_lowering=False,
        debug=False,
        enable_asserts=False,
        num_devices=N_CORES,
    )
    a_src = nc.dram_tensor("a_src", [36, T_SLABS * 128], bf, kind="ExternalInput").ap()
    b_src = nc.dram_tensor("b_src", [36, BEXT], bf, kind="ExternalInput").ap()
    p_src = nc.dram_tensor("p_src", [1, BEXT], bf, kind="ExternalInput").ap()
    pl_src = nc.dram_tensor("pl_src", [128, 2 * T_SLABS], bf, kind="ExternalInput").ap()
    # combined f32 output: cols [0:n_accum] = per-act-group accum row sums;
    # cols [n_accum:+512]: rows 0:2 = p-chain-A fold, row 64 = sw-chain-A fold
    out_f32 = nc.dram_tensor("out_f32", [128, n_accum], f32,
                             kind="ExternalOutput").ap()
    fold_f32 = nc.dram_tensor("fold_f32", [65, 512], f32,
                              kind="ExternalOutput").ap()
    tdd_out = nc.dram_tensor("tdd_out", [128, 512], i16, kind="ExternalOutput").ap()
    tda_out = nc.dram_tensor("tda_out", [128, 512], bf, kind="ExternalOutput").ap()

    with tile.TileContext(nc) as tc:
        with (
            tc.tile_pool(name="singles", bufs=1) as singles,
            tc.tile_pool(name="psA", bufs=1, space="PSUM") as psA_pool,
            tc.tile_pool(name="psB", bufs=1, space="PSUM") as psB_pool,
            tc.tile_pool(name="psD", bufs=1, space="PSUM") as psD_pool,
            tc.tile_pool(name="m2ps", bufs=1, space="PSUM") as m2_pool,
            tc.tile_pool(name="tpool", bufs=7) as t_pool,
            tc.tile_pool(name="trpool", bufs=6) as tr_pool,
            tc.tile_pool(name="upool", bufs=5) as u_pool,
        ):
            A = singles.tile([36, T_SLABS * 128], bf)
            B = singles.tile([36, BEXT], bf)
            P = singles.tile([128, BEXT], bf)
            PL = singles.tile([128, 2 * T_SLABS], bf)
            OUT = singles.tile([128, n_accum], f32)
            FOLD = singles.tile([65, 512], f32)
            ACCD = OUT[:, 0:n_accum]
            ONEC = singles.tile([128, 1], bf)
            M2 = m2_pool.tile([65, 512], f32)

            # t~0 warmups: ACT exp table load + PE ramp pin (adds 0 into M2,
            # and M2's first real chunk uses start=True anyway)
            DUM = singles.tile([128, 1], f32)
            DZ = singles.tile([1, 1], bf)
            nc.gpsimd.memset(DUM[:], 0.0)
            nc.gpsimd.memset(DZ[:], 0.0)
            nc.gpsimd.memset(OUT[:], 0.0)
            nc.gpsimd.memset(FOLD[:], 0.0)
            nc.gpsimd.memset(ONEC[:], 2.0 ** -SC)
            nc.scalar.activation(
                DUM[:], DUM[:], mybir.ActivationFunctionType.Exp, bias=0.0, scale=0.0
            )
            nc.tensor.matmul(M2[0:1, 0:1], lhsT=DZ[:], rhs=DZ[:], start=True,
                             stop=True, skip_group_check=True)

            # staged DMAs, ordered by first use (b cols for early groups, the
            # first p columns only when the lagged g0 contraction needs them)
            nc.sync.dma_start(A[:], a_src)
            nc.sync.dma_start(B[:, 128:640], b_src[:, 128:640])
            nc.sync.dma_start(B[:, 640:2688], b_src[:, 640:2688])
            nc.sync.dma_start(PL[:], pl_src)
            nc.sync.dma_start(P[:, 128:1152], p_src[:, 128:1152].broadcast_to((128, 1024)))
            nc.sync.dma_start(B[:, 2688:5248], b_src[:, 2688:5248])
            nc.sync.dma_start(P[:, 1152:2688], p_src[:, 1152:2688].broadcast_to((128, 1536)))
            nc.sync.dma_start(P[:, 2688:5248], p_src[:, 2688:5248].broadcast_to((128, 2560)))
            nc.sync.dma_start(B[:, 5248:7808], b_src[:, 5248:7808])
            nc.sync.dma_start(P[:, 5248:7808], p_src[:, 5248:7808].broadcast_to((128, 2560)))
            nc.sync.dma_start(B[:, 7808:10368], b_src[:, 7808:10368])
            nc.sync.dma_start(P[:, 7808:10368], p_src[:, 7808:10368].broadcast_to((128, 2560)))
            nc.sync.dma_start(B[:, 10368:12928], b_src[:, 10368:12928])
            nc.sync.dma_start(P[:, 10368:12928], p_src[:, 10368:12928].broadcast_to((128, 2560)))

            # contraction matmuls are issued with a lag so a late P/TR never
            # head-of-line-blocks the next group's z matmuls on PE; near the
            # end the lag drains so the tail chain stays short
            LAG = 4
            cons = [0, 0]           # p-chain, sw-chain counters
            n_cons = [n_conA, n_swA]
            trs = {}
            us = {}
            acc_i = 0

            def _chain_mm(ci, r0, nrow, cell, lhsT_fn, rhs_fn, segs_,
                          col_base=0):
                for (t, bcol, off, w) in _chunks(segs_, cell):
                    c0 = col_base + off % cell
                    nc.tensor.matmul(
                        M2[r0:r0 + nrow, c0:c0 + w],
                        lhsT=lhsT_fn(t),
                        rhs=rhs_fn(off, w),
                        start=cons[ci] == 0,
                        stop=cons[ci] == n_cons[ci] - 1,
                        skip_group_check=True,
                    )
                    cons[ci] += 1

            def _contract(g):
                (gi_, nb_, par_, segs_, typ_) = plan[g]
                TRg = trs.pop(g)
                _chain_mm(0, 0, 2, 512,
                          lambda t: PL[:, 2 * t:2 * t + 2],
                          lambda off, w: TRg[:, off:off + w], segs_)
                if typ_ == "dve":
                    Ug = us.pop(g)
                    # sw chain at partition 64 (96 is not a legal matmul output
                    # base), folded into cols [256:512] with cell 256
                    _chain_mm(1, 64, 1, 256,
                              lambda t: ONEC[:, 0:1],
                              lambda off, w: Ug[:, off:off + w].bitcast(bf),
                              segs_, col_base=256)
                if gi_ == ng - n_dump - 1:
                    # all chains finished: stage the folds on the (now idle)
                    # ACT engine while the dump groups still compute, and ship
                    # them in a small separate DMA
                    nc.scalar.activation(FOLD[0:2, :], M2[0:2, :],
                                         mybir.ActivationFunctionType.Copy,
                                         bias=0.0, scale=1.0)
                    nc.scalar.activation(FOLD[64:65, 256:512], M2[64:65, 256:512],
                                         mybir.ActivationFunctionType.Copy,
                                         bias=0.0, scale=1.0)
                    nc.sync.dma_start(fold_f32, FOLD[:])

            for (gi, nb, parity, segs, typ) in plan:
                width = nb * 128
                dump = gi >= ng - n_dump
                if parity == 2:
                    pt = psD_pool.tile([128, DVE_CAP * 128], f32, tag="psD")
                else:
                    pool_g = psA_pool if parity == 0 else psB_pool
                    pt = pool_g.tile([128, GROUP_CAPS[parity] * 128], f32,
                                     tag=f"ps{parity}")
                for (t, bcol, off, w) in _chunks(segs):
                    nc.tensor.matmul(
                        pt[:, off:off + w],
                        lhsT=A[:, t * 128:(t + 1) * 128],
                        rhs=B[:, bcol:bcol + w],
                        start=True,
                        stop=True,
                    )
                if typ == "act":
                    T = t_pool.tile([128, max(GROUP_CAPS) * 128], bf, tag="T")
                    kw = {} if dump else {"accum_out": ACCD[:, acc_i:acc_i + 1]}
                    nc.scalar.activation(
                        T[:, 0:width],
                        pt[:, 0:width],
                        mybir.ActivationFunctionType.Exp,
                        bias=0.0,
                        scale=1.0,
                        **kw,
                    )
                    if not dump:
                        acc_i += 1
                    Wsrc = lambda off, w: T[:, off:off + w]
                else:
                    U = u_pool.tile([128, max(GROUP_CAPS) * 128], i16, tag="U")
                    nc.vector.tensor_scalar(
                        U[:, 0:width], pt[:, 0:width], CVT_C1, CVT_C2,
                        mybir.AluOpType.mult, mybir.AluOpType.add,
                    )
                    if not dump:
                        us[gi] = U
                    Wsrc = lambda off, w: U[:, off:off + w].bitcast(bf)
                if dump:
                    # drain pending contractions, then ship the raw tile; the
                    # dve dump lands first (overlapping the final act group)
                    for g in sorted(trs):
                        _contract(g)
                    if typ == "dve":
                        nc.sync.dma_start(tdd_out, U[:, 0:width])
                    else:
                        nc.sync.dma_start(tda_out, T[:, 0:width])
                    continue
                if gi == ng - n_dump - 1:
                    nc.sync.dma_start(out_f32, OUT[:])
                TR = tr_pool.tile([128, max(GROUP_CAPS) * 128], bf, tag="TR")
                trs[gi] = TR
                for (off, pc, w) in _pslices(segs):
                    nc.vector.tensor_tensor(
                        TR[:, off:off + w], Wsrc(off, w), P[:, pc:pc + w],
                        mybir.AluOpType.mult,
                    )
                for g in sorted(trs):
                    if g <= gi - LAG or gi >= ng - n_dump - 2:
                        _contract(g)

            assert cons == n_cons, (cons, n_cons)
            assert acc_i == n_accum

    nc.compile()
    return nc


def _limbs3(x):
    x = np.asarray(x, np.float64)
    l1 = x.astype(BF16)
    r = x - l1.astype(np.float64)
    l2 = r.astype(BF16)
    r -= l2.astype(np.float64)
    l3 = r.astype(BF16)
    return l1, l2, l3


def _features(input, image):
    s = np.asarray(input, np.float32).reshape(N)
    img = np.asarray(image, np.float32).reshape(3, N)
    yy, xx = np.meshgrid(
        np.arange(H, dtype=np.float32), np.arange(W, dtype=np.float32), indexing="ij"
    )
    pos = np.stack([xx, yy], -1).reshape(N, 2) / np.float32(SIGMA_XY)
    feat = np.concatenate([pos, img.T / np.float32(SIGMA_RGB)], 1).astype(np.float32)
    return s, feat


def _prep_inputs(input, image):
    s, feat = _features(input, image)
    sq = (feat * feat).sum(1, dtype=np.float32)
    p = s.astype(np.float64) - 0.5

    fA, fB, fC = _limbs3(feat.T)
    t1, t2, t3 = _limbs3(-0.5 * sq.astype(np.float64))
    sq1, sq2, sq3 = _limbs3(sq)
    one = np.ones(N, BF16)
    half = np.full(N, -0.5, BF16)
    a = np.concatenate(
        [fA, fA, fB, fA, fC, fB, sq1[None], sq2[None], sq3[None],
         one[None], one[None], one[None]], axis=0).astype(BF16)
    b = np.concatenate(
        [fA, fB, fA, fC, fA, fB, half[None], half[None], half[None],
         t1[None], t2[None], t3[None]], axis=0).astype(BF16)
    p1 = p.astype(BF16)
    p2 = (p - p1.astype(np.float64)).astype(BF16)

    in_maps = []
    for k in range(N_CORES):
        own_rows = np.concatenate(
            [np.arange(((k + 8 * t) % NSLAB) * 128, ((k + 8 * t) % NSLAB) * 128 + 128)
             for t in range(T_SLABS)])
        # extended rotated columns: phys col slab m (1..100) -> global (k+m)%72
        bcols = np.concatenate(
            [np.arange(((k + m) % NSLAB) * 128, ((k + m) % NSLAB) * 128 + 128)
             for m in range(BEXT // 128)])
        # extended p columns; slabs m >= M0 (the DVE path) carry p * 2^-SC to
        # cancel the 2^SC scale of the bitcast exp
        pvec = p1[bcols].astype(np.float64)
        pvec[M0 * 128:] *= 2.0 ** -SC
        pl = np.stack([p1[own_rows].reshape(T_SLABS, 128),
                       p2[own_rows].reshape(T_SLABS, 128)], 1)   # [9, 2, 128]
        in_maps.append(
            {
                "a_src": np.ascontiguousarray(a[:, own_rows]),
                "b_src": np.ascontiguousarray(b[:, bcols]),
                "p_src": np.ascontiguousarray(pvec.astype(BF16))[None, :],
                "pl_src": np.ascontiguousarray(
                    pl.reshape(T_SLABS * 2, 128).T.astype(BF16)),
            }
        )
    return in_maps


def _host_corrections(input, image):
    """Exact f64 terms: + self blocks (d=0), - duplicate d=36 pair sums."""
    s, feat = _features(input, image)
    s64 = s.astype(np.float64)
    f64 = feat.astype(np.float64)
    total = 0.0
    for a0 in range(NSLAB):
        rows = slice(a0 * 128, a0 * 128 + 128)
        d2 = ((f64[rows][:, None, :] - f64[rows][None, :, :]) ** 2).sum(-1)
        Wm = np.exp(-0.5 * np.maximum(d2, 0.0))
        total += (s64[rows][:, None] * Wm * (1.0 - s64[rows])[None, :]).sum()
    for a0 in range(36):
        rows = slice(a0 * 128, a0 * 128 + 128)
        cols = slice((a0 + 36) * 128, (a0 + 36) * 128 + 128)
        d2 = ((f64[rows][:, None, :] - f64[cols][None, :, :]) ** 2).sum(-1)
        Wm = np.exp(-0.5 * np.maximum(d2, 0.0))
        pr = s64[rows] - 0.5
        pc = s64[cols] - 0.5
        total -= 0.5 * Wm.sum() - 2.0 * (pr @ Wm @ pc)
    return total


def _run(in_maps, **kwargs):
    if "nc" not in _cached:
        _cached["nc"] = _build_module()
    return bass_utils.run_bass_kernel_spmd(
        _cached["nc"], in_maps, core_ids=list(range(N_CORES)), **kwargs
    )


def kernel(input, image):
    assert input.shape == (1, 1, H, W) and image.shape == (1, 3, H, W)
    in_maps = _prep_inputs(input, image)
    res = _run(in_maps)

    s, feat = _features(input, image)
    p64 = s.astype(np.float64) - 0.5
    plan = _plan()
    n_accum = sum(1 for g in plan[:-1] if g[4] == "act")

    def dump_sum(k, td, segs):
        sub = 0.0
        off = 0
        for (t, d0, nblk, _o) in segs:
            rows = np.arange(((k + 8 * t) % NSLAB) * 128,
                             ((k + 8 * t) % NSLAB) * 128 + 128)
            for j in range(nblk):
                g = (k + 8 * t + d0 + j) % NSLAB
                cols = np.arange(g * 128, g * 128 + 128)
                Wb = td[:, off:off + 128]
                sub += 0.5 * Wb.sum() - 2.0 * (p64[rows] @ Wb @ p64[cols])
                off += 128
        return sub

    total = 0.0
    for k in range(N_CORES):
        r = res.results[k]
        total += 0.5 * r["out_f32"].sum(dtype=np.float64)
        fo = r["fold_f32"]
        total += 0.5 * fo[64, :].sum(dtype=np.float64)
        total -= 2.0 * fo[0:2, :].sum(dtype=np.float64)
        # host reduction of the two dumped tail groups
        total += dump_sum(
            k, r["tdd_out"].view(BF16).astype(np.float64) * 2.0 ** -SC,
            plan[-2][3])
        total += dump_sum(k, r["tda_out"].astype(np.float64), plan[-1][3])
    total += _host_corrections(input, image)
    return np.array(total / N, dtype=np.float32)
